# revision 1
# baseline (speedup 1.0000x reference)
"""Trainium2 Bass kernel for nn_Attention_8083128451525 (sparse_attention).

Strategy (validated against reference by golden-model + HW probes):
  - data-parallel: core b computes batch element b (B=8, 8 cores), no collectives
  - all matmuls in float32r (tf32-like, 1 cycle/row at N>=256), fp32 storage
  - 2D rope applied via stream_shuffle (pair swap) + 2 muls + 2 half-adds
  - decomposed rel-pos bias folded into ONE K=128 augmented S^T matmul:
      K~ = [roped k | onehot_h*8 | onehot_w*8],  Q~ = [roped q | U^T | V^T]
    where U[i,h'] = rq[i].rel_pos_h[h_i-h'+31], built by a P = RhT.T @ rq
    matmul + DRAM-bounce gather-DMA with overlapping access pattern.
    exp uses scale=1/8 so onehots are pre-scaled by 8.
  - softmax denominator = ones-column folded into augmented V (PV matmul
    row 64); normalization via reciprocal + 0-stride DMA broadcast.
  - qkv/proj biases folded as K=1 ones-row matmuls or ACT per-partition bias.
"""

import os
import sys

for _p in ("/opt/trn_rl_repo", "/root/.axon_site/_ro/trn_rl_repo"):
    if os.path.isdir(_p) and _p not in sys.path:
        sys.path.insert(0, _p)

import json
from contextlib import ExitStack

import numpy as np

import bass_rust
import concourse.bass as bass
import concourse.tile as tile
from concourse import mybir
from concourse.bass_utils import run_bass_kernel_spmd

F32R = mybir.dt.float32r
F32 = mybir.dt.float32
BF16 = mybir.dt.bfloat16
AF = mybir.ActivationFunctionType

NH, HD, HH, WW = 12, 64, 32, 32
L = HH * WW          # 1024
DIM = NH * HD        # 768
ROPE_THETA = 10000.0

# ---------------------------------------------------------------------------
# BIR post-fix: hoist excess semaphore waits onto injected NoOps (walrus
# instruction structs have limited wait slots; f32r Matmult rejects >1).
# ---------------------------------------------------------------------------
_nop_counter = [0]


def _transform_bir(json_bytes: bytes) -> bytes:
    data = json.loads(json_bytes)
    for fn in data.get("functions", []):
        for blk in fn.get("blocks", []):
            out = []
            for ins in blk.get("instructions", []):
                si = ins.get("sync_info")
                waits = (si or {}).get("on_wait") or []
                if len(waits) > 1:
                    keep = waits[-1:]
                    for w in waits[:-1]:
                        _nop_counter[0] += 1
                        out.append({
                            "name": f"I-birfix-{_nop_counter[0]}",
                            "opcode": "NoOp",
                            "engine": ins.get("engine"),
                            "ins": [],
                            "outs": [],
                            "sync_info": {"on_wait": [w], "on_update": []},
                        })
                    si["on_wait"] = keep
                out.append(ins)
            blk["instructions"] = out
    return json.dumps(data).encode()


def _install_birfix():
    if getattr(bass.Bass, "_birfix_installed", False):
        return
    orig = bass.Bass.to_json_bytes

    def patched(self, *a, **kw):
        return _transform_bir(orig(self, *a, **kw))

    bass.Bass.to_json_bytes = patched
    bass.Bass._birfix_installed = True


_install_birfix()


def _vp(pairs):
    return bass_rust.VecI64Pair(pairs)


def _cap(ap, pairs, offset=None):
    ap = ap.copy()
    ap.ap = _vp(pairs)
    if offset is not None:
        ap.offset = offset
    return ap


# ---------------------------------------------------------------------------
# Host-side constant prep
# ---------------------------------------------------------------------------
def _rope_angles():
    f = 1.0 / (ROPE_THETA ** (np.arange(0, HD, 4)[: HD // 4].astype(np.float32) / HD))
    t = np.arange(L)
    tx = (t % WW).astype(np.float32)
    ty = (t // WW).astype(np.float32)
    return np.concatenate([np.outer(tx, f), np.outer(ty, f)], axis=-1)  # (L, 32)


def _host_prep(qkv_w, qkv_b, proj_w, proj_b, rel_pos_h, rel_pos_w):
    cols, bias = [], []
    for n in range(NH):
        cols.append(qkv_w[0 * DIM + n * HD : 0 * DIM + (n + 1) * HD])
        cols.append(qkv_w[1 * DIM + n * HD : 1 * DIM + (n + 1) * HD])
        bias.append(qkv_b[0 * DIM + n * HD : 0 * DIM + (n + 1) * HD])
        bias.append(qkv_b[1 * DIM + n * HD : 1 * DIM + (n + 1) * HD])
    W_t = np.ascontiguousarray(np.concatenate(cols, axis=0).T)        # (768, 1536)
    b_qk = np.concatenate(bias).reshape(NH, 128).T.copy()             # (128, 12)

    Wv_t = np.ascontiguousarray(qkv_w[2 * DIM :].T)                   # (768, 768)
    bv_row = qkv_b[2 * DIM :].reshape(1, DIM).copy()
    P_t = np.ascontiguousarray(proj_w.T)                              # (768, 768)
    pb_row = proj_b.reshape(1, DIM).copy()

    ang = _rope_angles()
    cos, sin = np.cos(ang), np.sin(ang)                               # (L, 32)
    CC = np.empty((128, L), np.float32)
    SS = np.empty((128, L), np.float32)
    for p in range(128):
        m = (p % 64) // 2
        CC[p] = cos[:, m]
        SS[p] = sin[:, m] if (p % 2) else -sin[:, m]

    j = np.arange(L)
    OHOW = np.zeros((64, L), np.float32)
    for p in range(32):
        OHOW[p] = 8.0 * ((j >> 5) == (31 - p))
        OHOW[32 + p] = 8.0 * ((j & 31) == (31 - p))

    # padded to 64 output rows (M=64) so downstream copies stay aligned
    RhT = np.zeros((HD, 64), np.float32)
    RhT[:, 0:63] = rel_pos_h.T
    RwT = np.zeros((HD, 64), np.float32)
    RwT[:, 0:63] = rel_pos_w.T

    vtmpl = np.zeros((128, 780), np.float32)
    vtmpl[:, 64::65] = 1.0
    import ml_dtypes
    vtmplh = vtmpl.astype(ml_dtypes.bfloat16)
    ones_row = np.ones((1, 128), np.float32)

    return dict(W_t=W_t, b_qk=b_qk, Wv_t=Wv_t, bv_row=bv_row, P_t=P_t,
                pb_row=pb_row, CC=CC, SS=SS, OHOW=OHOW, RhT=RhT, RwT=RwT,
                vtmpl=vtmpl, vtmplh=vtmplh, ones_row=ones_row)


# ---------------------------------------------------------------------------
# Bass program (one core, one batch element)
# ---------------------------------------------------------------------------
def build_bass(iters=1, gps=False, ebf16=False, allbf16=False, pv2=True,
               sbf16=False, ts2=True, t22=False, deep=True, wq5=True):
    if allbf16:
        ebf16 = True
    nc = bass.Bass()

    DT = BF16 if allbf16 else F32R
    SDT = BF16 if (sbf16 or allbf16) else F32R
    xT = nc.declare_dram_parameter("xT", [DIM, L], DT, isOutput=False)
    W_t = nc.declare_dram_parameter("W_t", [DIM, 1536], DT, isOutput=False)
    b_qk = nc.declare_dram_parameter("b_qk", [128, NH], F32, isOutput=False)
    Wv_t = nc.declare_dram_parameter("Wv_t", [DIM, DIM], DT, isOutput=False)
    bv_row = nc.declare_dram_parameter("bv_row", [1, DIM], DT, isOutput=False)
    P_t = nc.declare_dram_parameter("P_t", [DIM, DIM], DT, isOutput=False)
    pb_row = nc.declare_dram_parameter("pb_row", [1, DIM], DT, isOutput=False)
    CCd = nc.declare_dram_parameter("CC", [128, L], F32, isOutput=False)
    SSd = nc.declare_dram_parameter("SS", [128, L], F32, isOutput=False)
    OHOWd = nc.declare_dram_parameter("OHOW", [64, L], SDT, isOutput=False)
    RhTd = nc.declare_dram_parameter("RhT", [HD, 64], SDT, isOutput=False)
    RwTd = nc.declare_dram_parameter("RwT", [HD, 64], SDT, isOutput=False)
    vtmpl = nc.declare_dram_parameter("vtmpl", [128, 780], F32R, isOutput=False)
    vtmplh = nc.declare_dram_parameter("vtmplh", [128, 780], BF16, isOutput=False)
    ones_d = nc.declare_dram_parameter("ones_row", [1, 128], DT, isOutput=False)
    outD = nc.declare_dram_parameter("out", [L, DIM], F32R, isOutput=True)

    with tile.TileContext(nc) as tc:
        est = ExitStack()
        consts = est.enter_context(tc.tile_pool(name="consts", bufs=1))
        xtp = est.enter_context(tc.tile_pool(name="xtp", bufs=1))
        wqp = est.enter_context(tc.tile_pool(name="wqp", bufs=(5 if wq5 else 4) if deep else 3))
        vwp = est.enter_context(tc.tile_pool(name="vwp", bufs=1))
        ripool = est.enter_context(tc.tile_pool(name="ripool", bufs=2))
        vap = est.enter_context(tc.tile_pool(name="vap", bufs=1))
        qap = est.enter_context(tc.tile_pool(name="qap", bufs=3))
        kap = est.enter_context(tc.tile_pool(name="kap", bufs=3))
        scr = est.enter_context(tc.tile_pool(name="scr", bufs=1))
        tsp = est.enter_context(tc.tile_pool(name="tsp", bufs=2 if ts2 else 1))
        t2p = est.enter_context(tc.tile_pool(name="t2p", bufs=2 if t22 else 1))
        pps = est.enter_context(tc.tile_pool(name="pps", bufs=2))
        epool = est.enter_context(tc.tile_pool(name="epool", bufs=4 if deep else 3))
        misc = est.enter_context(tc.tile_pool(name="misc", bufs=2))
        recp = est.enter_context(tc.tile_pool(name="recp", bufs=2))
        outtp = est.enter_context(tc.tile_pool(name="outtp", bufs=1))
        osb = est.enter_context(tc.tile_pool(name="osb", bufs=1))
        dram = est.enter_context(tc.tile_pool(name="dram", bufs=6 if ts2 else 3, space="DRAM"))
        psum = est.enter_context(tc.tile_pool(name="psum", bufs=1 if pv2 else 2, space="PSUM"))
        psum_s = est.enter_context(tc.tile_pool(name="psum_s", bufs=2, space="PSUM"))
        psum_pv = est.enter_context(tc.tile_pool(name="psum_pv", bufs=2 if pv2 else 1, space="PSUM"))

        # ---- constants ----
        cc = consts.tile([128, L], F32, tag="cc")
        ss = consts.tile([128, L], F32, tag="ss")
        bqk = consts.tile([128, NH], F32, tag="bqk")
        bv = consts.tile([1, DIM], DT, tag="bv")
        pb = consts.tile([1, DIM], DT, tag="pb")
        rht = consts.tile([HD, 64], SDT, tag="rht")
        rwt = consts.tile([HD, 64], SDT, tag="rwt")
        ones1 = consts.tile([1, 128], DT, tag="ones1")
        nc.sync.dma_start(cc[:], CCd[:])
        nc.sync.dma_start(ss[:], SSd[:])
        nc.sync.dma_start(bqk[:], b_qk[:])
        nc.sync.dma_start(bv[:], bv_row[:])
        nc.sync.dma_start(pb[:], pb_row[:])
        nc.sync.dma_start(rht[:], RhTd[:])
        nc.sync.dma_start(rwt[:], RwTd[:])
        nc.sync.dma_start(ones1[:], ones_d[:])

        # ---- xT tiles (alive for qk + v matmuls) ----
        xts = []
        for k in range(6):
            t = xtp.tile([128, L], DT, tag=f"xt{k}")
            nc.sync.dma_start(t[:], xT[k * 128 : (k + 1) * 128, :])
            xts.append(t)

        for _it in range(iters):
            # ---- v matmul (natural orientation) + bias row -> vaug tiles ----
            vws = []
            for k in range(6):
                t = vwp.tile([128, DIM], DT, tag=f"vw{k}")
                nc.sync.dma_start(t[:], Wv_t[k * 128 : (k + 1) * 128, :])
                vws.append(t)
            vaug = []
            for m in range(8):
                vps = psum.tile([128, L], F32, tag="mm")
                for c0, cw in ((0, 512), (512, 256)):
                    for k in range(6):
                        nc.tensor.matmul(
                            vps[:, c0 : c0 + cw],
                            xts[k][:, m * 128 : (m + 1) * 128],
                            vws[k][:, c0 : c0 + cw],
                            start=(k == 0), stop=False,
                        )
                    nc.tensor.matmul(vps[:, c0 : c0 + cw], ones1[:, 0:128],
                                     bv[:, c0 : c0 + cw], start=False, stop=True)
                va = vap.tile([128, 780], BF16 if ebf16 else F32R, tag=f"va{m}")
                nc.sync.dma_start(va[:], vtmplh[:] if ebf16 else vtmpl[:])
                nc.scalar.activation(_cap(va[:], [[780, 128], [65, 8], [1, 64]]),
                                     vps[:, 0:512], AF.Copy)
                nc.scalar.activation(_cap(va[:], [[780, 128], [65, 4], [1, 64]], 65 * 8),
                                     vps[:, 512:768], AF.Copy)
                vaug.append(va)

            # ---- per-head pipeline ----
            outt = []
            for k in range(6):
                ot = outtp.tile([128, L], DT, tag=f"ot{k}", name=f"ot{k}")
                outt.append(ot)
            swap_mask = [i ^ 1 for i in range(32)]

            def prep(n):
                # qk matmul for head n: W m-tile n = [q_n | k_n] columns
                wm = wqp.tile([128, 768], DT, tag="wm", name="wm")
                nc.sync.dma_start(
                    wm[:], _cap(W_t[:], [[1536, 128], [128 * 1536, 6], [1, 128]], n * 128))
                qkps = psum.tile([128, L], F32, tag="mm", name="qkps")
                for ih in range(2):
                    for k in range(6):
                        nc.tensor.matmul(
                            qkps[:, ih * 512 : (ih + 1) * 512],
                            wm[:, k * 128 : (k + 1) * 128],
                            xts[k][:, ih * 512 : (ih + 1) * 512],
                            start=(k == 0), stop=(k == 5),
                        )
                ri = ripool.tile([128, L], F32, tag="ri", name="ri")
                nc.scalar.activation(ri[:], qkps[:], AF.Identity,
                                     bias=bqk[:, n : n + 1], scale=1.0)

                # rope
                ts_ = tsp.tile([128, L], F32, tag="ts", name="ts_")
                nc.vector.stream_shuffle(ts_[:], ri[:], swap_mask)
                t1 = scr.tile([128, L], F32, tag="t1", name="t1")
                nc.vector.tensor_mul(t1[:], ri[:], cc[:])
                t2 = t2p.tile([128, L], F32, tag="t2", name="t2")
                (nc.gpsimd if gps else nc.vector).tensor_mul(t2[:], ts_[:], ss[:])
                qa = qap.tile([128, L], SDT, tag="qa", name="qa")
                ka = kap.tile([128, L], SDT, tag="ka", name="ka")
                nc.vector.tensor_add(qa[0:64, :], t1[0:64, :], t2[0:64, :])
                nc.vector.tensor_add(ka[0:64, :], t1[64:128, :], t2[64:128, :])
                nc.sync.dma_start(ka[64:128, :], OHOWd[:])

                # rel-pos P matmuls on roped q (M padded to 64)
                php = psum.tile([128, L], F32, tag="mm", name="php")
                for ih in range(2):
                    nc.tensor.matmul(php[0:64, ih * 512 : (ih + 1) * 512],
                                     rht[:], qa[0:64, ih * 512 : (ih + 1) * 512],
                                     start=True, stop=True)
                phs = pps.tile([64, L], SDT, tag="phs", name="phs")
                nc.scalar.activation(phs[:], php[0:64, :], AF.Copy)
                phd = dram.tile([64, L], SDT, tag="phd", name="phd")
                nc.sync.dma_start(phd[:], phs[:])

                pwp = psum.tile([128, L], F32, tag="mm", name="pwp")
                for ih in range(2):
                    rhs = _cap(qa[0:64, :], [[L, 64], [1, 16], [32, 32]], ih * 16)
                    nc.tensor.matmul(pwp[0:64, ih * 512 : (ih + 1) * 512],
                                     rwt[:], rhs, start=True, stop=True)
                pws = pps.tile([64, L], SDT, tag="pws", name="pws")
                nc.scalar.activation(pws[:], pwp[0:64, :], AF.Copy)
                pwd = dram.tile([64, L], SDT, tag="pwd", name="pwd")
                nc.sync.dma_start(pwd[:], pws[:])

                # gather U^T into qa[64:96]; gather Vt (w-major) + unpermute
                nc.sync.dma_start(qa[64:96, :],
                                  _cap(phd[:], [[1024, 32], [1056, 32], [1, 32]]))
                vts = misc.tile([32, L], SDT, tag="vts", name="vts")
                nc.sync.dma_start(vts[:],
                                  _cap(pwd[:], [[1024, 32], [1056, 32], [1, 32]]))
                (nc.gpsimd if gps else nc.vector).tensor_copy(
                    _cap(qa[96:128, :], [[L, 32], [32, 32], [1, 32]]),
                    _cap(vts[:], [[L, 32], [1, 32], [32, 32]]))
                return qa, ka

            def attn(n, qa, ka):
                # S^T -> exp -> PV, one PSUM bank per (jt, ih)
                edt = BF16 if ebf16 else F32R
                pvp = psum_pv.tile([65, L], F32, tag="pv", name="pvp")
                for jt in range(8):
                    for ih in range(2):
                        sps = psum_s.tile([128, 512], F32, tag="s", name="sps")
                        nc.tensor.matmul(sps[:],
                                         ka[:, jt * 128 : (jt + 1) * 128],
                                         qa[:, ih * 512 : (ih + 1) * 512],
                                         start=True, stop=True)
                        ej = epool.tile([128, 512], edt, tag="ej", name="ej")
                        nc.scalar.activation(ej[:], sps[:], AF.Exp, scale=0.125)
                        nc.tensor.matmul(pvp[:, ih * 512 : (ih + 1) * 512],
                                         vaug[jt][:, 65 * n : 65 * n + 65],
                                         ej[:],
                                         start=(jt == 0), stop=(jt == 7))

                # normalize via reciprocal + 0-stride DMA broadcast
                rec = recp.tile([1, L], F32, tag="rec", name="rec")
                nc.vector.reciprocal(rec[:], pvp[64:65, :])
                recd = dram.tile([1, L], F32, tag="recd", name="recd")
                nc.sync.dma_start(recd[:], rec[:])
                rec64 = misc.tile([64, L], F32, tag="rec64", name="rec64")
                nc.sync.dma_start(rec64[:], _cap(recd[:], [[0, 64], [1, L]]))
                nc.vector.tensor_mul(outt[n // 2][(n % 2) * 64 : (n % 2) * 64 + 64, :],
                                     pvp[0:64, :], rec64[:])

            state = {0: prep(0), 1: prep(1)}
            for n in range(NH):
                if n + 2 < NH:
                    state[n + 2] = prep(n + 2)
                qa, ka = state.pop(n)
                attn(n, qa, ka)

            # ---- proj ----
            pts = []
            for k in range(6):
                t = vwp.tile([128, DIM], DT, tag=f"vw{k}")
                nc.sync.dma_start(t[:], P_t[k * 128 : (k + 1) * 128, :])
                pts.append(t)
            for m in range(8):
                prp = psum.tile([128, L], F32, tag="mm")
                for c0, cw in ((0, 512), (512, 256)):
                    for k in range(6):
                        nc.tensor.matmul(
                            prp[:, c0 : c0 + cw],
                            outt[k][:, m * 128 : (m + 1) * 128],
                            pts[k][:, c0 : c0 + cw],
                            start=(k == 0), stop=False,
                        )
                    nc.tensor.matmul(prp[:, c0 : c0 + cw], ones1[:, 0:128],
                                     pb[:, c0 : c0 + cw], start=False, stop=True)
                ob = osb.tile([128, DIM], F32R, tag="ob")
                nc.scalar.activation(ob[:], prp[:, 0:768], AF.Copy)
                nc.sync.dma_start(outD[m * 128 : (m + 1) * 128, :], ob[:])


        est.close()
    return nc


BEST_FLAGS = dict(gps=True, ebf16=True, allbf16=False)

_BF16_KEYS = ("W_t", "Wv_t", "bv_row", "P_t", "pb_row", "OHOW", "RhT", "RwT",
              "ones_row")


def _convert_maps(C, xT_all, allbf16, sbf16=False):
    import ml_dtypes
    C = dict(C)
    if sbf16 and not allbf16:
        for k in ("OHOW", "RhT", "RwT"):
            C[k] = C[k].astype(ml_dtypes.bfloat16)
        return C, xT_all
    if not allbf16:
        return C, xT_all
    for k in _BF16_KEYS:
        C[k] = C[k].astype(ml_dtypes.bfloat16)
    return C, xT_all.astype(ml_dtypes.bfloat16)

_BUILT = None


def _get_built():
    global _BUILT
    if _BUILT is None:
        _BUILT = build_bass(1, **BEST_FLAGS)
    return _BUILT


def _ensure_axon():
    """Re-enable the axon backend if the caller pinned JAX_PLATFORMS=cpu
    (common in reference harnesses)."""
    import jax

    def has_axon():
        try:
            return any(getattr(d, "platform", "") == "axon" or "NC_" in str(d)
                       for d in jax.devices())
        except Exception:
            return False

    if has_axon():
        return
    os.environ.pop("JAX_PLATFORMS", None)
    try:
        jax.config.update("jax_platforms", None)
    except Exception:
        pass
    try:
        from jax._src import xla_bridge
        xla_bridge._clear_backends()
    except Exception:
        pass
    assert has_axon(), "axon/neuron devices not visible to jax"


def kernel(x, qkv_w, qkv_b, proj_w, proj_b, rel_pos_h, rel_pos_w):
    _ensure_axon()
    x = np.asarray(x, np.float32)
    B = x.shape[0]
    C = _host_prep(np.asarray(qkv_w, np.float32), np.asarray(qkv_b, np.float32),
                   np.asarray(proj_w, np.float32), np.asarray(proj_b, np.float32),
                   np.asarray(rel_pos_h, np.float32), np.asarray(rel_pos_w, np.float32))
    xT_all = np.ascontiguousarray(x.reshape(B, L, DIM).transpose(0, 2, 1))
    C, xT_all = _convert_maps(C, xT_all, BEST_FLAGS.get("allbf16", False), BEST_FLAGS.get("sbf16", False))

    nc = _get_built()
    in_maps = [dict(C, xT=xT_all[b]) for b in range(B)]
    res = run_bass_kernel_spmd(nc, in_maps, list(range(B))).results
    out = np.stack([res[b]["out"] for b in range(B)])  # (B, 1024, 768)
    return np.ascontiguousarray(out.reshape(B, HH, WW, DIM).astype(np.float32))



# revision 80
# speedup vs baseline: 1.0936x; 1.0936x over previous
"""Trainium2 Bass kernel for nn_Attention_8083128451525 (sparse_attention).

Strategy (v2 — pipelined rewrite of the validated v1 algorithm):
  - data-parallel: core b computes batch element b (B=8, 8 cores), no collectives
  - all weight/x matmuls in bf16 (PE 1 cyc/row, same as f32r, half the DMA);
    per-head qk weight blocks pre-packed host-side for contiguous DMA; qkv/v/
    proj biases folded as K=1 rank-1 matmuls (no ACT bias copies)
  - 2D rope via stream_shuffle (pair swap) + muls + half-adds, reading the qk
    PSUM directly; roped q/k stored bf16
  - decomposed rel-pos bias folded into ONE K=128 augmented S^T matmul:
      K~ = [roped k | onehot_h*8 | onehot_w*8],  Q~ = [roped q | U^T | V^T]
    U/V built by a single merged [Rh|Rw]^T @ rq matmul; the staging copy
    writes the w half in w-major order so both gather DMAs have contiguous
    inner dims; one Pool unpermute restores query order for V^T.
  - softmax denominator = ones-column folded into augmented V (PV row 64);
    normalize = PSUM evacuation + reciprocal + DRAM-bounce broadcast + Pool
    multiply (last head: K=1 matmul broadcast, no DRAM round-trip).
  - PSUM as 8 banks: mm 2x[128,512] (qkv/relpos/v/proj), s 2x[128,1024]
    (score chunks, exp reads 1024 wide), pv 2x[128,512] (PV per query-half).
  - software pipelining, depth 3: during head n the chunk loop interleaves
    head n+3's qk matmuls (3/chunk, jt0-4), rope halves at jt2/jt4, head
    n+2's rel-pos matmuls + staging at jt5/jt7, with the S pair emitted one
    chunk ahead so the PE never idles on the exp (ACT) engine.
"""

import os
import sys

for _p in ("/opt/trn_rl_repo", "/root/.axon_site/_ro/trn_rl_repo"):
    if os.path.isdir(_p) and _p not in sys.path:
        sys.path.insert(0, _p)

import json
from contextlib import ExitStack

import numpy as np

import bass_rust
import concourse.bass as bass
import concourse.tile as tile
from concourse import mybir
from concourse.bass_utils import run_bass_kernel_spmd

F32R = mybir.dt.float32r
F32 = mybir.dt.float32
BF16 = mybir.dt.bfloat16
AF = mybir.ActivationFunctionType

NH, HD, HH, WW = 12, 64, 32, 32
L = HH * WW          # 1024
DIM = NH * HD        # 768
ROPE_THETA = 10000.0

# ---------------------------------------------------------------------------
# BIR post-fix: hoist excess semaphore waits onto injected NoOps (walrus
# instruction structs have limited wait slots; f32r Matmult rejects >1).
# ---------------------------------------------------------------------------
_nop_counter = [0]


def _transform_bir(json_bytes: bytes) -> bytes:
    data = json.loads(json_bytes)
    for fn in data.get("functions", []):
        for blk in fn.get("blocks", []):
            out = []
            for ins in blk.get("instructions", []):
                si = ins.get("sync_info")
                waits = (si or {}).get("on_wait") or []
                if len(waits) > 1:
                    keep = waits[-1:]
                    for w in waits[:-1]:
                        _nop_counter[0] += 1
                        out.append({
                            "name": f"I-birfix-{_nop_counter[0]}",
                            "opcode": "NoOp",
                            "engine": ins.get("engine"),
                            "ins": [],
                            "outs": [],
                            "sync_info": {"on_wait": [w], "on_update": []},
                        })
                    si["on_wait"] = keep
                out.append(ins)
            blk["instructions"] = out
    return json.dumps(data).encode()


def _install_birfix():
    if getattr(bass.Bass, "_birfix_installed", False):
        return
    orig = bass.Bass.to_json_bytes

    def patched(self, *a, **kw):
        return _transform_bir(orig(self, *a, **kw))

    bass.Bass.to_json_bytes = patched
    bass.Bass._birfix_installed = True


_install_birfix()


def _vp(pairs):
    return bass_rust.VecI64Pair(pairs)


def _cap(ap, pairs, offset=None):
    ap = ap.copy()
    ap.ap = _vp(pairs)
    if offset is not None:
        ap.offset = offset
    return ap


# ---------------------------------------------------------------------------
# Host-side constant prep
# ---------------------------------------------------------------------------
def _rope_angles():
    f = 1.0 / (ROPE_THETA ** (np.arange(0, HD, 4)[: HD // 4].astype(np.float32) / HD))
    t = np.arange(L)
    tx = (t % WW).astype(np.float32)
    ty = (t // WW).astype(np.float32)
    return np.concatenate([np.outer(tx, f), np.outer(ty, f)], axis=-1)  # (L, 32)


def _host_prep(qkv_w, qkv_b, proj_w, proj_b, rel_pos_h, rel_pos_w):
    import ml_dtypes

    # qk weights per head, pre-packed for contiguous per-head DMA:
    # W_h[n*128:(n+1)*128, k*128+c] = W[k*128 + p, head-n col block]
    cols, brows = [], []
    for n in range(NH):
        qn = qkv_w[0 * DIM + n * HD : 0 * DIM + (n + 1) * HD]   # (64, 768)
        kn = qkv_w[1 * DIM + n * HD : 1 * DIM + (n + 1) * HD]
        blk = np.concatenate([qn, kn], axis=0).T                # (768, 128)
        # reshape to [128 part, 768]: wm[p, k*128+c] = blk[k*128+p, c]
        wm = np.concatenate([blk[k * 128 : (k + 1) * 128] for k in range(6)],
                            axis=1)                             # (128, 768)
        cols.append(wm)
        brows.append(np.concatenate(
            [qkv_b[0 * DIM + n * HD : 0 * DIM + (n + 1) * HD],
             qkv_b[1 * DIM + n * HD : 1 * DIM + (n + 1) * HD]]))
    W_h = np.concatenate(cols, axis=0)                          # (1536, 768)
    bqkr = np.concatenate(brows).reshape(1, NH * 128)           # (1, 1536)

    Wv_t = np.ascontiguousarray(qkv_w[2 * DIM :].T)             # (768, 768)
    bv_row = qkv_b[2 * DIM :].reshape(1, DIM).copy()
    P_t = np.ascontiguousarray(proj_w.T)                        # (768, 768)
    pb_row = proj_b.reshape(1, DIM).copy()

    ang = _rope_angles()
    cos, sin = np.cos(ang), np.sin(ang)                         # (L, 32)
    CC = np.empty((128, L), np.float32)
    SS = np.empty((128, L), np.float32)
    for p in range(128):
        m = (p % 64) // 2
        CC[p] = cos[:, m]
        SS[p] = sin[:, m] if (p % 2) else -sin[:, m]

    j = np.arange(L)
    OHOW = np.zeros((64, L), np.float32)
    for p in range(32):
        OHOW[p] = 8.0 * ((j >> 5) == (31 - p))
        OHOW[32 + p] = 8.0 * ((j & 31) == (31 - p))

    # merged rel-pos lhsT [64, 128]: cols 0-63 = Rh^T (63 rows padded to 64),
    # cols 64-127 = Rw^T
    RHW = np.zeros((HD, 128), np.float32)
    RHW[:, 0:63] = rel_pos_h.T
    RHW[:, 64:127] = rel_pos_w.T

    vtmpl = np.zeros((128, 780), np.float32)
    vtmpl[:, 64::65] = 1.0
    vtmplh = vtmpl.astype(ml_dtypes.bfloat16)

    bf = ml_dtypes.bfloat16
    # v-bias passes through attention unchanged (softmax rows sum to 1), so
    # fold it into the proj bias: pb' = pb + proj_w @ bv.
    pbf = proj_b + proj_w @ qkv_b[2 * DIM :]
    # single-row constant blob: [bqkr(1536) | (unused 768) | pb'(768) | ones(512)]
    blob = np.concatenate(
        [bqkr.ravel(), np.zeros(DIM, np.float32), pbf, np.ones(512, np.float32)]
    ).reshape(1, -1).astype(bf)
    return dict(
        W_h=W_h.astype(bf), cblob=blob,
        Wv_t=Wv_t.astype(bf), P_t=P_t.astype(bf),
        CC=CC, SS=SS, OHOW=OHOW.astype(bf), RHW=RHW.astype(bf),
        vtmplh=vtmplh)


def make_inmaps(x, qkv_w, qkv_b, proj_w, proj_b, rel_pos_h, rel_pos_w):
    import ml_dtypes

    x = np.asarray(x, np.float32)
    B = x.shape[0]
    C = _host_prep(np.asarray(qkv_w, np.float32), np.asarray(qkv_b, np.float32),
                   np.asarray(proj_w, np.float32), np.asarray(proj_b, np.float32),
                   np.asarray(rel_pos_h, np.float32),
                   np.asarray(rel_pos_w, np.float32))
    xT_all = np.ascontiguousarray(
        x.reshape(B, L, DIM).transpose(0, 2, 1)).astype(ml_dtypes.bfloat16)
    return [dict(C, xT=xT_all[b]) for b in range(B)]


# ---------------------------------------------------------------------------
# Bass program (one core, one batch element)
# ---------------------------------------------------------------------------
def build_bass(iters=1, **flags):
    nc = bass.Bass()

    xT = nc.declare_dram_parameter("xT", [DIM, L], BF16, isOutput=False)
    W_hd = nc.declare_dram_parameter("W_h", [12 * 128, DIM], BF16, isOutput=False)
    cblobd = nc.declare_dram_parameter("cblob", [1, 3584], BF16, isOutput=False)
    Wv_t = nc.declare_dram_parameter("Wv_t", [DIM, DIM], BF16, isOutput=False)
    P_t = nc.declare_dram_parameter("P_t", [DIM, DIM], BF16, isOutput=False)
    CCd = nc.declare_dram_parameter("CC", [128, L], F32, isOutput=False)
    SSd = nc.declare_dram_parameter("SS", [128, L], F32, isOutput=False)
    OHOWd = nc.declare_dram_parameter("OHOW", [64, L], BF16, isOutput=False)
    RHWd = nc.declare_dram_parameter("RHW", [HD, 128], BF16, isOutput=False)
    vtmplh = nc.declare_dram_parameter("vtmplh", [128, 780], BF16, isOutput=False)
    outD = nc.declare_dram_parameter("out", [L, DIM], BF16, isOutput=True)

    with tile.TileContext(nc) as tc:
        est = ExitStack()
        consts = est.enter_context(tc.tile_pool(name="consts", bufs=1))
        xtp = est.enter_context(tc.tile_pool(name="xtp", bufs=1))
        vwp = est.enter_context(tc.tile_pool(name="vwp", bufs=1))
        vap = est.enter_context(tc.tile_pool(name="vap", bufs=1))
        wqp = est.enter_context(tc.tile_pool(name="wqp", bufs=4))
        qap = est.enter_context(tc.tile_pool(name="qap", bufs=4))
        kap = est.enter_context(tc.tile_pool(name="kap", bufs=4))
        t1p = est.enter_context(tc.tile_pool(name="t1p", bufs=2))
        tsp = est.enter_context(tc.tile_pool(name="tsp", bufs=2))
        t2p = est.enter_context(tc.tile_pool(name="t2p", bufs=2))
        phsp = est.enter_context(tc.tile_pool(name="phsp", bufs=2))
        vtsp = est.enter_context(tc.tile_pool(name="vtsp", bufs=2))
        ejp = est.enter_context(tc.tile_pool(name="ejp", bufs=6))
        recp = est.enter_context(tc.tile_pool(name="recp", bufs=4))
        rec64p = est.enter_context(tc.tile_pool(name="rec64p", bufs=4))
        pvcp = est.enter_context(tc.tile_pool(name="pvcp", bufs=4))
        outtp = est.enter_context(tc.tile_pool(name="outtp", bufs=1))
        obp = est.enter_context(tc.tile_pool(name="obp", bufs=3))
        dram = est.enter_context(tc.tile_pool(name="dram", bufs=3, space="DRAM"))
        ps = est.enter_context(tc.tile_pool(name="ps", bufs=4, space="PSUM"))

        # ---- x / v-weight tiles first (earliest first matmul), then consts --
        xts, vws0 = [], []
        for k in range(6):
            t = xtp.tile([128, L], BF16, tag=f"xt{k}")
            nc.sync.dma_start(t[:], xT[k * 128 : (k + 1) * 128, :])
            xts.append(t)
            w = vwp.tile([128, DIM], BF16, tag=f"vw{k}")
            nc.sync.dma_start(w[:], Wv_t[k * 128 : (k + 1) * 128, :])
            vws0.append(w)

        cblobt = consts.tile([1, 3584], BF16, tag="cblob")
        rhw = consts.tile([HD, 128], BF16, tag="rhw")
        nc.sync.dma_start(cblobt[:], cblobd[:])
        nc.sync.dma_start(rhw[:], RHWd[:])

        def cs(a, b):
            return cblobt[0:1, a:b]

        BV, PB, ONES = 1536, 2304, 3072

        # prefetch per-head weights + onehots for the prologue heads 0-2
        wm_pre, ka_pre = {}, {}
        for n in range(3):
            wm = wqp.tile([128, DIM], BF16, tag="wm", name=f"wmp{n}")
            nc.sync.dma_start(wm[:], W_hd[n * 128 : (n + 1) * 128, :])
            wm_pre[n] = wm
            ka = kap.tile([128, L], BF16, tag="ka", name=f"kap{n}")
            nc.sync.dma_start(ka[64:128, :], OHOWd[:])
            ka_pre[n] = ka

        cc = consts.tile([128, L], F32, tag="cc")
        ss = consts.tile([128, L], F32, tag="ss")
        nc.sync.dma_start(cc[:], CCd[:])
        nc.sync.dma_start(ss[:], SSd[:])

        swap_mask = [i ^ 1 for i in range(32)]
        # PSUM: mm 2x1 bank + s 2x2 banks + pv 2x1 bank = 8 banks
        PS_BUFS = {"mm": 2, "pv": 2}

        def ps_tile(tag, name):
            return ps.tile([128, 512], F32, tag=tag, bufs=PS_BUFS[tag],
                           name=name)

        def ps_big(name):
            return ps.tile([128, 1024], F32, tag="s", bufs=2, name=name)

        for _it in range(iters):
            if _it == 0:
                vws = vws0
            else:
                # proj reused the vw tags last iteration; reload
                vws = []
                for k in range(6):
                    w = vwp.tile([128, DIM], BF16, tag=f"vw{k}")
                    nc.sync.dma_start(w[:], Wv_t[k * 128 : (k + 1) * 128, :])
                    vws.append(w)
            # ---- augmented-V tiles + templates (issued on the ACT queue so
            # they don't serialize behind the x/weight stream on SP) ----
            vaug = []
            for m in range(8):
                va = vap.tile([128, 780], BF16, tag=f"va{m}", name=f"va{m}")
                nc.scalar.dma_start(va[:], vtmplh[:])
                vaug.append(va)

            outt = []
            for k in range(6):
                ot = outtp.tile([128, L], BF16, tag=f"ot{k}", name=f"ot{k}")
                outt.append(ot)

            # ---------------- per-head prep machinery ----------------
            qa_t, ka_t = {}, {}

            def prep_begin(n, qk_tag, phw_tag):
                if n in wm_pre:
                    wm = wm_pre.pop(n)
                    ka = ka_pre.pop(n)
                else:
                    # per-head weight/onehot loads ride the ACT DMA queue so
                    # the SP queue stays clear for the gather chain
                    wm = wqp.tile([128, DIM], BF16, tag="wm", name="wm")
                    nc.sync.dma_start(wm[:], W_hd[n * 128 : (n + 1) * 128, :])
                    ka = kap.tile([128, L], BF16, tag="ka", name="ka")
                    nc.sync.dma_start(ka[64:128, :], OHOWd[:])
                ka_t[n] = ka
                if qk_tag == "s":
                    qkt = ps_big(f"qk{n}")
                    qk = [qkt[:, 0:512], qkt[:, 512:1024]]
                else:
                    qk = [ps_tile(qk_tag, f"qk{n}_{ih}")[:] for ih in range(2)]
                ctx = dict(n=n, ka=ka, qk=qk, phw_tag=phw_tag, phw=[])
                pe = []
                for ih in range(2):
                    # k0 carries start; bias second; k5 carries stop
                    def mk(ih, k, start, stop):
                        return lambda: nc.tensor.matmul(
                            qk[ih], wm[:, k * 128 : (k + 1) * 128],
                            xts[k][:, ih * 512 : (ih + 1) * 512],
                            start=start, stop=stop)

                    pe.append(mk(ih, 0, True, False))
                    pe.append(lambda ih=ih: nc.tensor.matmul(
                        qk[ih], cs(n * 128, (n + 1) * 128), cs(ONES, ONES + 512),
                        start=False, stop=False))
                    for k in range(1, 6):
                        pe.append(mk(ih, k, False, k == 5))

                def phw_mm(ih):
                    t = ps_tile(ctx["phw_tag"], f"phw{ih}")
                    ctx["phw"].append(t)
                    qa = ctx["qa"]
                    nc.tensor.matmul(t[:], rhw[:],
                                     qa[0:64, ih * 512 : (ih + 1) * 512],
                                     start=True, stop=True)

                pe.append(lambda: phw_mm(0))
                pe.append(lambda: phw_mm(1))
                ctx["pe"] = pe
                return ctx

            def rope_half(ctx, ih):
                n, qk, ka = ctx["n"], ctx["qk"], ctx["ka"]
                if ih == 0:
                    ctx["t1"] = t1p.tile([128, L], BF16, tag="t1", name="t1")
                    ctx["ts"] = tsp.tile([128, L], F32, tag="ts", name="ts_")
                    ctx["t2"] = t2p.tile([128, L], BF16, tag="t2", name="t2")
                    qa = qap.tile([128, L], BF16, tag="qa", name="qa")
                    ctx["qa"] = qa
                    qa_t[n] = qa
                t1, ts_, t2, qa = ctx["t1"], ctx["ts"], ctx["t2"], ctx["qa"]
                sl = slice(ih * 512, (ih + 1) * 512)
                nc.vector.tensor_mul(t1[:, sl], qk[ih], cc[:, sl])
                nc.vector.stream_shuffle(ts_[:, sl], qk[ih], swap_mask)
                nc.vector.tensor_mul(t2[:, sl], ts_[:, sl], ss[:, sl])
                nc.vector.tensor_add(qa[0:64, sl], t1[0:64, sl], t2[0:64, sl])
                nc.gpsimd.tensor_add(ka[0:64, sl], t1[64:128, sl],
                                     t2[64:128, sl])

            def prep_copy(ctx, ih):
                # PSUM -> SBUF staging of phw half ih, split so it runs as
                # soon as that phw matmul lands (keeps the mm ring moving)
                if ih == 0:
                    ctx["phs"] = phsp.tile([128, L], BF16, tag="phs",
                                           name="phs")
                phs = ctx["phs"]
                off64 = phs[64:128, :].offset
                # ph half: natural query order (Pool can't read PSUM, so
                # ACT for half 0, DVE for half 1 to spread the load)
                nc.scalar.activation(
                    phs[0:64, ih * 512 : (ih + 1) * 512],
                    ctx["phw"][ih][0:64, :], AF.Copy)
                # pw half: written w-major (col w*32+h) so the V gather
                # below has a contiguous inner dim (DVE)
                nc.vector.tensor_copy(
                    _cap(phs[:], [[1024, 64], [1, 16], [32, 32]],
                         off64 + 16 * ih),
                    ctx["phw"][ih][64:128, :])

            def prep_end(ctx):
                qa, phs = ctx["qa"], ctx["phs"]
                phd = dram.tile([128, L], BF16, tag="phd", name="phd")
                nc.sync.dma_start(phd[:], phs[:])
                # U^T gather: qa[64+p, i] = ph[h_i + p, i]
                nc.sync.dma_start(
                    qa[64:96, :],
                    _cap(phd[:], [[1024, 32], [1056, 32], [1, 32]]))
                # V^T gather from w-major pw rows: vts[p, w*32+h] = pw[w+p, (h,w)]
                vts = vtsp.tile([32, L], BF16, tag="vts", name="vts")
                nc.sync.dma_start(
                    vts[:],
                    _cap(phd[:], [[1024, 32], [1056, 32], [1, 32]], 64 * 1024))
                # unpermute w-major -> natural into qa[96:128]
                nc.gpsimd.tensor_copy(
                    _cap(qa[96:128, :], [[L, 32], [32, 32], [1, 32]]),
                    _cap(vts[:], [[L, 32], [1, 32], [32, 32]]))

            def prep_qkri(n, qk_tag, phw_tag):
                ctx = prep_begin(n, qk_tag, phw_tag)
                for f in ctx["pe"][:7]:
                    f()
                rope_half(ctx, 0)
                for f in ctx["pe"][7:14]:
                    f()
                rope_half(ctx, 1)
                return ctx

            def prep_phw_end(ctx):
                ctx["pe"][14]()
                prep_copy(ctx, 0)
                ctx["pe"][15]()
                prep_copy(ctx, 1)
                prep_end(ctx)

            # ---------------- v phase (pairs of m-tiles; mm+pv rings give a
            # 4-deep pipe since attention hasn't started yet) ----
            def v_pass(groups, wtiles, sink):
                slots = [ps_tile("mm" if i % 2 == 0 else "pv", f"vs{i}")
                         for i in range(len(groups))]
                for k in range(6):
                    for (m, ch), slot in zip(groups, slots):
                        c0, cw = (0, 512) if ch == 0 else (512, 256)
                        nc.tensor.matmul(
                            slot[:, 0:cw],
                            xts[k][:, m * 128 : (m + 1) * 128],
                            wtiles[k][:, c0 : c0 + cw],
                            start=(k == 0), stop=(k == 5))
                for (m, ch), slot in zip(groups, slots):
                    sink(m, ch, slot)

            def va_sink(m, ch, slot):
                if ch == 0:
                    nc.scalar.activation(
                        _cap(vaug[m][:], [[780, 128], [65, 8], [1, 64]]),
                        slot[:, 0:512], AF.Copy)
                else:
                    nc.scalar.activation(
                        _cap(vaug[m][:], [[780, 128], [65, 4], [1, 64]], 65 * 8),
                        slot[:, 0:256], AF.Copy)

            vp = ([[(m, 0), (m + 1, 0)] for m in (0, 2, 4, 6)]
                  + [[(m, 1), (m + 1, 1)] for m in (0, 2, 4, 6)])
            preps = {}
            v_pass(vp[0], vws, va_sink)
            v_pass(vp[1], vws, va_sink)
            p0 = prep_qkri(0, "s", "pv")
            v_pass(vp[2], vws, va_sink)
            v_pass(vp[3], vws, va_sink)
            prep_phw_end(p0)
            p1 = prep_qkri(1, "s", "pv")
            v_pass(vp[4], vws, va_sink)
            v_pass(vp[5], vws, va_sink)
            prep_phw_end(p1)
            preps[2] = prep_qkri(2, "s", "mm")
            v_pass(vp[6], vws, va_sink)
            v_pass(vp[7], vws, va_sink)

            # ---------------- attention head loop (8 chunks per head) -------
            def S_pair(n, jt):
                sb = ps_big("sb")
                for ih in range(2):
                    nc.tensor.matmul(
                        sb[:, ih * 512 : (ih + 1) * 512],
                        ka_t[n][:, jt * 128 : (jt + 1) * 128],
                        qa_t[n][:, ih * 512 : (ih + 1) * 512],
                        start=True, stop=True)
                return sb

            def normalize(n, ih, pvt):
                last = n == NH - 1
                # evacuate PSUM immediately (frees the pv bank for the next
                # head); the rest runs from SBUF
                src = pvcp.tile([65, 512], F32, tag="pvc", name="pvc")
                nc.vector.tensor_copy(src[:], pvt[0:65, :])
                out_ap = outt[n // 2][(n % 2) * 64 : (n % 2) * 64 + 64,
                                      ih * 512 : (ih + 1) * 512]
                if last:
                    # tail latency path: broadcast the reciprocal row via a
                    # K=1 matmul instead of the DRAM round-trip, multiply on
                    # DVE (reads one PSUM operand); bf16 so dtypes match the
                    # ones row
                    recb = recp.tile([1, 512], BF16, tag="recb", name="recb")
                    with nc.allow_low_precision(reason="bf16 recip broadcast"):
                        nc.vector.reciprocal(recb[:], src[64:65, :])
                    recP = ps_tile("mm", "recP")
                    nc.tensor.matmul(recP[0:64, :], cs(ONES, ONES + 64),
                                     recb[:], start=True, stop=True)
                    nc.vector.tensor_mul(out_ap, src[0:64, :], recP[0:64, :])
                else:
                    rec = recp.tile([1, 512], F32, tag="rec", name="rec")
                    nc.vector.reciprocal(rec[:], src[64:65, :])
                    # partition-broadcast via DRAM bounce, issued on the ACT
                    # queue (SEQ-side cost only; keeps the SP queue clear for
                    # the gather chain)
                    recd = dram.tile([1, 512], F32, tag="recd", bufs=4,
                                     name="recd")
                    nc.sync.dma_start(recd[:], rec[:])
                    rec64 = rec64p.tile([64, 512], F32, tag="rec64",
                                        name="rec64")
                    nc.sync.dma_start(rec64[:], _cap(recd[:],
                                                     [[0, 64], [1, 512]]))
                    nc.gpsimd.tensor_mul(out_ap, src[0:64, :], rec64[:])

            pending = S_pair(0, 0)
            pvt = [None, None]
            pts = []
            for n in range(NH):
                if n in (8, 9, 10):
                    # prefetch proj weights (2 per head) while attention
                    # finishes
                    for k in range(2 * (n - 8), 2 * (n - 8) + 2):
                        t = vwp.tile([128, DIM], BF16, tag=f"vw{k}")
                        nc.sync.dma_start(t[:], P_t[k * 128 : (k + 1) * 128, :])
                        pts.append(t)
                ctx_qk = preps[n + 3] = prep_begin(n + 3, "mm", "mm") \
                    if n + 3 < NH else None
                ctx_ph = preps.get(n + 2)
                for jt in range(8):
                    sb = pending
                    ej = ejp.tile([128, 1024], BF16, tag="ej", name="ej")
                    nc.scalar.activation(ej[:], sb[:], AF.Exp, scale=0.125)
                    if jt < 7:
                        pending = S_pair(n, jt + 1)
                    elif n + 1 < NH:
                        pending = S_pair(n + 1, 0)
                    if jt == 0:
                        pvt[0] = ps_tile("pv", "pvA")
                        pvt[1] = ps_tile("pv", "pvB")
                    for ih in range(2):
                        nc.tensor.matmul(
                            pvt[ih][0:65, :],
                            vaug[jt][:, 65 * n : 65 * n + 65],
                            ej[:, ih * 512 : (ih + 1) * 512],
                            start=(jt == 0), stop=(jt == 7))
                    if ctx_qk is not None and jt <= 3:
                        # 4 qk matmuls per chunk -> both halves stopped by jt3
                        for i in range(4 * jt, min(4 * jt + 4, 14)):
                            ctx_qk["pe"][i]()
                        if jt == 1:
                            rope_half(ctx_qk, 0)
                        if jt == 3:
                            rope_half(ctx_qk, 1)
                    if jt == 4 and ctx_ph is not None:
                        ctx_ph["pe"][14]()
                        prep_copy(ctx_ph, 0)
                    if jt == 6 and ctx_ph is not None:
                        ctx_ph["pe"][15]()
                        prep_copy(ctx_ph, 1)
                    if jt == 7:
                        normalize(n, 0, pvt[0])
                        normalize(n, 1, pvt[1])
                        if ctx_ph is not None:
                            prep_end(ctx_ph)

            # ---------------- proj (alternate s/mm rings for 4-deep pipe) ---
            for m in range(8):
                ob = obp.tile([128, DIM], BF16, tag="ob", name="ob")
                if m % 3 == 0:
                    big = ps_big("prs")
                    slots = [big[:, 0:512], big[:, 512:768]]
                elif m % 3 == 1:
                    slots = [ps_tile("mm", "prsA")[:, 0:512],
                             ps_tile("mm", "prsA")[:, 0:256]]
                else:
                    slots = [ps_tile("pv", "prsB")[:, 0:512],
                             ps_tile("pv", "prsB")[:, 0:256]]
                for ch in range(2):
                    c0, cw = (0, 512) if ch == 0 else (512, 256)
                    slot = slots[ch]
                    for k in range(6):
                        nc.tensor.matmul(
                            slot,
                            outt[k][:, m * 128 : (m + 1) * 128],
                            pts[k][:, c0 : c0 + cw],
                            start=(k == 0), stop=False)
                    nc.tensor.matmul(slot, cs(ONES, ONES + 128),
                                     cs(PB + c0, PB + c0 + cw),
                                     start=False, stop=True)
                    nc.scalar.activation(ob[:, c0 : c0 + cw], slot, AF.Copy)
                nc.sync.dma_start(outD[m * 128 : (m + 1) * 128, :], ob[:])

        est.close()
    return nc


BEST_FLAGS = dict()

_BUILT = None


def _get_built():
    global _BUILT
    if _BUILT is None:
        _BUILT = build_bass(1, **BEST_FLAGS)
    return _BUILT


def _ensure_axon():
    """Re-enable the axon backend if the caller pinned JAX_PLATFORMS=cpu
    (common in reference harnesses)."""
    import jax

    def has_axon():
        try:
            return any(getattr(d, "platform", "") == "axon" or "NC_" in str(d)
                       for d in jax.devices())
        except Exception:
            return False

    if has_axon():
        return
    os.environ.pop("JAX_PLATFORMS", None)
    try:
        jax.config.update("jax_platforms", None)
    except Exception:
        pass
    try:
        from jax._src import xla_bridge
        xla_bridge._clear_backends()
    except Exception:
        pass
    assert has_axon(), "axon/neuron devices not visible to jax"


def kernel(x, qkv_w, qkv_b, proj_w, proj_b, rel_pos_h, rel_pos_w):
    _ensure_axon()
    x = np.asarray(x, np.float32)
    B = x.shape[0]
    in_maps = make_inmaps(x, qkv_w, qkv_b, proj_w, proj_b,
                          rel_pos_h, rel_pos_w)
    nc = _get_built()
    res = run_bass_kernel_spmd(nc, in_maps, list(range(B))).results
    out = np.stack([res[b]["out"] for b in range(B)])  # (B, 1024, 768)
    return np.ascontiguousarray(out.reshape(B, HH, WW, DIM).astype(np.float32))


# revision 82
# speedup vs baseline: 1.1272x; 1.0307x over previous
"""Trainium2 Bass kernel for nn_Attention_8083128451525 (sparse_attention).

Strategy (v2 — pipelined rewrite of the validated v1 algorithm):
  - data-parallel: core b computes batch element b (B=8, 8 cores), no collectives
  - all weight/x matmuls in bf16 (PE 1 cyc/row, same as f32r, half the DMA);
    per-head qk weight blocks pre-packed host-side for contiguous DMA; qk/
    proj biases folded as K=1 rank-1 matmuls; v-bias folded into the proj
    bias on the host (softmax rows sum to 1, so bv passes through attention)
  - 2D rope via stream_shuffle (pair swap) + muls + half-adds, reading the qk
    PSUM directly; roped q/k stored bf16
  - decomposed rel-pos bias folded into ONE K=128 augmented S^T matmul:
      K~ = [roped k | onehot_h*8 | onehot_w*8],  Q~ = [roped q | U^T | V^T]
    U/V built by a single merged [Rh|Rw]^T @ rq matmul; the staging copy
    writes the w half in w-major order so both gather DMAs have contiguous
    inner dims; one Pool unpermute restores query order for V^T.
  - softmax denominator = ones-column folded into augmented V (PV row 64);
    normalize = PSUM evacuation + reciprocal + DRAM-bounce broadcast + Pool
    multiply (last head: K=1 matmul broadcast, no DRAM round-trip).
  - PSUM as 8 banks: mm 2x[128,512] (qkv/relpos/v/proj), s 2x[128,1024]
    (score chunks, exp reads 1024 wide), pv 2x[128,512] (PV per query-half).
  - software pipelining, depth 3: during head n the chunk loop interleaves
    head n+3's qk matmuls (3/chunk, jt0-4), rope halves at jt2/jt4, head
    n+2's rel-pos matmuls + staging at jt5/jt7, with the S pair emitted one
    chunk ahead so the PE never idles on the exp (ACT) engine.
"""

import os
import sys

for _p in ("/opt/trn_rl_repo", "/root/.axon_site/_ro/trn_rl_repo"):
    if os.path.isdir(_p) and _p not in sys.path:
        sys.path.insert(0, _p)

import json
from contextlib import ExitStack

import numpy as np

import bass_rust
import concourse.bass as bass
import concourse.tile as tile
from concourse import mybir
from concourse.bass_utils import run_bass_kernel_spmd

F32R = mybir.dt.float32r
F32 = mybir.dt.float32
BF16 = mybir.dt.bfloat16
AF = mybir.ActivationFunctionType

NH, HD, HH, WW = 12, 64, 32, 32
L = HH * WW          # 1024
DIM = NH * HD        # 768
ROPE_THETA = 10000.0

# ---------------------------------------------------------------------------
# BIR post-fix: hoist excess semaphore waits onto injected NoOps (walrus
# instruction structs have limited wait slots; f32r Matmult rejects >1).
# ---------------------------------------------------------------------------
_nop_counter = [0]


def _transform_bir(json_bytes: bytes) -> bytes:
    data = json.loads(json_bytes)
    for fn in data.get("functions", []):
        for blk in fn.get("blocks", []):
            out = []
            for ins in blk.get("instructions", []):
                si = ins.get("sync_info")
                waits = (si or {}).get("on_wait") or []
                if len(waits) > 1:
                    keep = waits[-1:]
                    for w in waits[:-1]:
                        _nop_counter[0] += 1
                        out.append({
                            "name": f"I-birfix-{_nop_counter[0]}",
                            "opcode": "NoOp",
                            "engine": ins.get("engine"),
                            "ins": [],
                            "outs": [],
                            "sync_info": {"on_wait": [w], "on_update": []},
                        })
                    si["on_wait"] = keep
                out.append(ins)
            blk["instructions"] = out
    return json.dumps(data).encode()


def _install_birfix():
    if getattr(bass.Bass, "_birfix_installed", False):
        return
    orig = bass.Bass.to_json_bytes

    def patched(self, *a, **kw):
        return _transform_bir(orig(self, *a, **kw))

    bass.Bass.to_json_bytes = patched
    bass.Bass._birfix_installed = True


_install_birfix()


def _vp(pairs):
    return bass_rust.VecI64Pair(pairs)


def _cap(ap, pairs, offset=None):
    ap = ap.copy()
    ap.ap = _vp(pairs)
    if offset is not None:
        ap.offset = offset
    return ap


# ---------------------------------------------------------------------------
# Host-side constant prep
# ---------------------------------------------------------------------------
def _rope_angles():
    f = 1.0 / (ROPE_THETA ** (np.arange(0, HD, 4)[: HD // 4].astype(np.float32) / HD))
    t = np.arange(L)
    tx = (t % WW).astype(np.float32)
    ty = (t // WW).astype(np.float32)
    return np.concatenate([np.outer(tx, f), np.outer(ty, f)], axis=-1)  # (L, 32)


def _host_prep(qkv_w, qkv_b, proj_w, proj_b, rel_pos_h, rel_pos_w):
    import ml_dtypes

    # qk weights per head, pre-packed for contiguous per-head DMA:
    # W_h[n*128:(n+1)*128, k*128+c] = W[k*128 + p, head-n col block]
    cols, brows = [], []
    for n in range(NH):
        qn = qkv_w[0 * DIM + n * HD : 0 * DIM + (n + 1) * HD]   # (64, 768)
        kn = qkv_w[1 * DIM + n * HD : 1 * DIM + (n + 1) * HD]
        blk = np.concatenate([qn, kn], axis=0).T                # (768, 128)
        # reshape to [128 part, 768]: wm[p, k*128+c] = blk[k*128+p, c]
        wm = np.concatenate([blk[k * 128 : (k + 1) * 128] for k in range(6)],
                            axis=1)                             # (128, 768)
        cols.append(wm)
        brows.append(np.concatenate(
            [qkv_b[0 * DIM + n * HD : 0 * DIM + (n + 1) * HD],
             qkv_b[1 * DIM + n * HD : 1 * DIM + (n + 1) * HD]]))
    W_h = np.concatenate(cols, axis=0)                          # (1536, 768)
    bqkr = np.concatenate(brows).reshape(1, NH * 128)           # (1, 1536)

    Wv_t = np.ascontiguousarray(qkv_w[2 * DIM :].T)             # (768, 768)
    bv_row = qkv_b[2 * DIM :].reshape(1, DIM).copy()
    P_t = np.ascontiguousarray(proj_w.T)                        # (768, 768)
    pb_row = proj_b.reshape(1, DIM).copy()

    ang = _rope_angles()
    cos, sin = np.cos(ang), np.sin(ang)                         # (L, 32)
    CC = np.empty((128, L), np.float32)
    SS = np.empty((128, L), np.float32)
    for p in range(128):
        m = (p % 64) // 2
        CC[p] = cos[:, m]
        SS[p] = sin[:, m] if (p % 2) else -sin[:, m]

    j = np.arange(L)
    OHOW = np.zeros((64, L), np.float32)
    for p in range(32):
        OHOW[p] = 8.0 * ((j >> 5) == (31 - p))
        OHOW[32 + p] = 8.0 * ((j & 31) == (31 - p))

    # merged rel-pos lhsT [64, 128]: cols 0-63 = Rh^T (63 rows padded to 64),
    # cols 64-127 = Rw^T
    RHW = np.zeros((HD, 128), np.float32)
    RHW[:, 0:63] = rel_pos_h.T
    RHW[:, 64:127] = rel_pos_w.T

    vtmpl = np.zeros((128, 780), np.float32)
    vtmpl[:, 64::65] = 1.0
    vtmplh = vtmpl.astype(ml_dtypes.bfloat16)

    bf = ml_dtypes.bfloat16
    # v-bias passes through attention unchanged (softmax rows sum to 1), so
    # fold it into the proj bias: pb' = pb + proj_w @ bv.
    pbf = proj_b + proj_w @ qkv_b[2 * DIM :]
    # single-row constant blob: [bqkr(1536) | (unused 768) | pb'(768) | ones(512)]
    blob = np.concatenate(
        [bqkr.ravel(), np.zeros(DIM, np.float32), pbf, np.ones(512, np.float32)]
    ).reshape(1, -1).astype(bf)
    return dict(
        W_h=W_h.astype(bf), cblob=blob,
        Wv_t=Wv_t.astype(bf), P_t=P_t.astype(bf),
        CC=CC, SS=SS, OHOW=OHOW.astype(bf), RHW=RHW.astype(bf),
        vtmplh=vtmplh)


def make_inmaps(x, qkv_w, qkv_b, proj_w, proj_b, rel_pos_h, rel_pos_w):
    import ml_dtypes

    x = np.asarray(x, np.float32)
    B = x.shape[0]
    C = _host_prep(np.asarray(qkv_w, np.float32), np.asarray(qkv_b, np.float32),
                   np.asarray(proj_w, np.float32), np.asarray(proj_b, np.float32),
                   np.asarray(rel_pos_h, np.float32),
                   np.asarray(rel_pos_w, np.float32))
    xT_all = np.ascontiguousarray(
        x.reshape(B, L, DIM).transpose(0, 2, 1)).astype(ml_dtypes.bfloat16)
    return [dict(C, xT=xT_all[b]) for b in range(B)]


# ---------------------------------------------------------------------------
# Bass program (one core, one batch element)
# ---------------------------------------------------------------------------
def build_bass(iters=1, **flags):
    nc = bass.Bass()

    xT = nc.declare_dram_parameter("xT", [DIM, L], BF16, isOutput=False)
    W_hd = nc.declare_dram_parameter("W_h", [12 * 128, DIM], BF16, isOutput=False)
    cblobd = nc.declare_dram_parameter("cblob", [1, 3584], BF16, isOutput=False)
    Wv_t = nc.declare_dram_parameter("Wv_t", [DIM, DIM], BF16, isOutput=False)
    P_t = nc.declare_dram_parameter("P_t", [DIM, DIM], BF16, isOutput=False)
    CCd = nc.declare_dram_parameter("CC", [128, L], F32, isOutput=False)
    SSd = nc.declare_dram_parameter("SS", [128, L], F32, isOutput=False)
    OHOWd = nc.declare_dram_parameter("OHOW", [64, L], BF16, isOutput=False)
    RHWd = nc.declare_dram_parameter("RHW", [HD, 128], BF16, isOutput=False)
    vtmplh = nc.declare_dram_parameter("vtmplh", [128, 780], BF16, isOutput=False)
    outD = nc.declare_dram_parameter("out", [L, DIM], BF16, isOutput=True)

    with tile.TileContext(nc) as tc:
        est = ExitStack()
        consts = est.enter_context(tc.tile_pool(name="consts", bufs=1))
        xtp = est.enter_context(tc.tile_pool(name="xtp", bufs=1))
        vwp = est.enter_context(tc.tile_pool(name="vwp", bufs=1))
        vap = est.enter_context(tc.tile_pool(name="vap", bufs=1))
        wqp = est.enter_context(tc.tile_pool(name="wqp", bufs=4))
        qap = est.enter_context(tc.tile_pool(name="qap", bufs=4))
        kap = est.enter_context(tc.tile_pool(name="kap", bufs=4))
        t1p = est.enter_context(tc.tile_pool(name="t1p", bufs=2))
        tsp = est.enter_context(tc.tile_pool(name="tsp", bufs=2))
        t2p = est.enter_context(tc.tile_pool(name="t2p", bufs=2))
        phsp = est.enter_context(tc.tile_pool(name="phsp", bufs=2))
        vtsp = est.enter_context(tc.tile_pool(name="vtsp", bufs=2))
        ejp = est.enter_context(tc.tile_pool(name="ejp", bufs=6))
        recp = est.enter_context(tc.tile_pool(name="recp", bufs=4))
        rec64p = est.enter_context(tc.tile_pool(name="rec64p", bufs=4))
        pvcp = est.enter_context(tc.tile_pool(name="pvcp", bufs=4))
        outtp = est.enter_context(tc.tile_pool(name="outtp", bufs=1))
        obp = est.enter_context(tc.tile_pool(name="obp", bufs=3))
        dram = est.enter_context(tc.tile_pool(name="dram", bufs=3, space="DRAM"))
        ps = est.enter_context(tc.tile_pool(name="ps", bufs=4, space="PSUM"))

        # ---- x / v-weight tiles first (earliest first matmul), then consts --
        xts, vws0 = [], []
        for k in range(6):
            t = xtp.tile([128, L], BF16, tag=f"xt{k}")
            nc.sync.dma_start(t[:], xT[k * 128 : (k + 1) * 128, :])
            xts.append(t)
            w = vwp.tile([128, DIM], BF16, tag=f"vw{k}")
            nc.sync.dma_start(w[:], Wv_t[k * 128 : (k + 1) * 128, :])
            vws0.append(w)

        cblobt = consts.tile([1, 3584], BF16, tag="cblob")
        rhw = consts.tile([HD, 128], BF16, tag="rhw")
        nc.sync.dma_start(cblobt[:], cblobd[:])
        nc.sync.dma_start(rhw[:], RHWd[:])

        def cs(a, b):
            return cblobt[0:1, a:b]

        BV, PB, ONES = 1536, 2304, 3072

        # prefetch per-head weights + onehots for the prologue heads 0-2
        wm_pre, ka_pre = {}, {}
        for n in range(3):
            wm = wqp.tile([128, DIM], BF16, tag="wm", name=f"wmp{n}")
            nc.sync.dma_start(wm[:], W_hd[n * 128 : (n + 1) * 128, :])
            wm_pre[n] = wm
            ka = kap.tile([128, L], BF16, tag="ka", name=f"kap{n}")
            nc.sync.dma_start(ka[64:128, :], OHOWd[:])
            ka_pre[n] = ka

        cc = consts.tile([128, L], F32, tag="cc")
        ss = consts.tile([128, L], F32, tag="ss")
        nc.sync.dma_start(cc[:], CCd[:])
        nc.sync.dma_start(ss[:], SSd[:])

        swap_mask = [i ^ 1 for i in range(32)]
        # PSUM: mm 2x1 bank + s 2x2 banks + pv 2x1 bank = 8 banks
        PS_BUFS = {"mm": 2, "pv": 2}

        def ps_tile(tag, name):
            return ps.tile([128, 512], F32, tag=tag, bufs=PS_BUFS[tag],
                           name=name)

        def ps_big(name):
            return ps.tile([128, 1024], F32, tag="s", bufs=2, name=name)

        for _it in range(iters):
            if _it == 0:
                vws = vws0
            else:
                # proj reused the vw tags last iteration; reload
                vws = []
                for k in range(6):
                    w = vwp.tile([128, DIM], BF16, tag=f"vw{k}")
                    nc.sync.dma_start(w[:], Wv_t[k * 128 : (k + 1) * 128, :])
                    vws.append(w)
            # ---- augmented-V tiles + templates (issued on the ACT queue so
            # they don't serialize behind the x/weight stream on SP) ----
            vaug = []
            for m in range(8):
                va = vap.tile([128, 780], BF16, tag=f"va{m}", name=f"va{m}")
                nc.scalar.dma_start(va[:], vtmplh[:])
                vaug.append(va)

            outt = []
            for k in range(6):
                ot = outtp.tile([128, L], BF16, tag=f"ot{k}", name=f"ot{k}")
                outt.append(ot)

            # ---------------- per-head prep machinery ----------------
            qa_t, ka_t = {}, {}

            def prep_begin(n, qk_tag, phw_tag):
                if n in wm_pre:
                    wm = wm_pre.pop(n)
                    ka = ka_pre.pop(n)
                else:
                    # per-head weight/onehot loads ride the ACT DMA queue so
                    # the SP queue stays clear for the gather chain
                    wm = wqp.tile([128, DIM], BF16, tag="wm", name="wm")
                    nc.sync.dma_start(wm[:], W_hd[n * 128 : (n + 1) * 128, :])
                    ka = kap.tile([128, L], BF16, tag="ka", name="ka")
                    nc.sync.dma_start(ka[64:128, :], OHOWd[:])
                ka_t[n] = ka
                if qk_tag == "s":
                    qkt = ps_big(f"qk{n}")
                    qk = [qkt[:, 0:512], qkt[:, 512:1024]]
                else:
                    qk = [ps_tile(qk_tag, f"qk{n}_{ih}")[:] for ih in range(2)]
                ctx = dict(n=n, ka=ka, qk=qk, phw_tag=phw_tag, phw=[])
                pe = []
                for ih in range(2):
                    # k0 carries start; bias second; k5 carries stop
                    def mk(ih, k, start, stop):
                        return lambda: nc.tensor.matmul(
                            qk[ih], wm[:, k * 128 : (k + 1) * 128],
                            xts[k][:, ih * 512 : (ih + 1) * 512],
                            start=start, stop=stop)

                    pe.append(mk(ih, 0, True, False))
                    pe.append(lambda ih=ih: nc.tensor.matmul(
                        qk[ih], cs(n * 128, (n + 1) * 128), cs(ONES, ONES + 512),
                        start=False, stop=False))
                    for k in range(1, 6):
                        pe.append(mk(ih, k, False, k == 5))

                def phw_mm(ih):
                    t = ps_tile(ctx["phw_tag"], f"phw{ih}")
                    ctx["phw"].append(t)
                    qa = ctx["qa"]
                    nc.tensor.matmul(t[:], rhw[:],
                                     qa[0:64, ih * 512 : (ih + 1) * 512],
                                     start=True, stop=True)

                pe.append(lambda: phw_mm(0))
                pe.append(lambda: phw_mm(1))
                ctx["pe"] = pe
                return ctx

            def rope_half(ctx, ih):
                n, qk, ka = ctx["n"], ctx["qk"], ctx["ka"]
                if ih == 0:
                    ctx["t1"] = t1p.tile([128, L], BF16, tag="t1", name="t1")
                    ctx["ts"] = tsp.tile([128, L], F32, tag="ts", name="ts_")
                    ctx["t2"] = t2p.tile([128, L], BF16, tag="t2", name="t2")
                    qa = qap.tile([128, L], BF16, tag="qa", name="qa")
                    ctx["qa"] = qa
                    qa_t[n] = qa
                t1, ts_, t2, qa = ctx["t1"], ctx["ts"], ctx["t2"], ctx["qa"]
                sl = slice(ih * 512, (ih + 1) * 512)
                nc.vector.tensor_mul(t1[:, sl], qk[ih], cc[:, sl])
                nc.vector.stream_shuffle(ts_[:, sl], qk[ih], swap_mask)
                nc.vector.tensor_mul(t2[:, sl], ts_[:, sl], ss[:, sl])
                nc.vector.tensor_add(qa[0:64, sl], t1[0:64, sl], t2[0:64, sl])
                nc.gpsimd.tensor_add(ka[0:64, sl], t1[64:128, sl],
                                     t2[64:128, sl])

            def prep_copy(ctx, ih):
                # PSUM -> SBUF staging of phw half ih, split so it runs as
                # soon as that phw matmul lands (keeps the mm ring moving)
                if ih == 0:
                    ctx["phs"] = phsp.tile([128, L], BF16, tag="phs",
                                           name="phs")
                phs = ctx["phs"]
                off64 = phs[64:128, :].offset
                # ph half: natural query order (Pool can't read PSUM, so
                # ACT for half 0, DVE for half 1 to spread the load)
                nc.scalar.activation(
                    phs[0:64, ih * 512 : (ih + 1) * 512],
                    ctx["phw"][ih][0:64, :], AF.Copy)
                # pw half: written w-major (col w*32+h) so the V gather
                # below has a contiguous inner dim (DVE)
                nc.vector.tensor_copy(
                    _cap(phs[:], [[1024, 64], [1, 16], [32, 32]],
                         off64 + 16 * ih),
                    ctx["phw"][ih][64:128, :])

            def prep_end(ctx):
                qa, phs = ctx["qa"], ctx["phs"]
                phd = dram.tile([128, L], BF16, tag="phd", name="phd")
                nc.sync.dma_start(phd[:], phs[:])
                # U^T gather: qa[64+p, i] = ph[h_i + p, i]
                nc.sync.dma_start(
                    qa[64:96, :],
                    _cap(phd[:], [[1024, 32], [1056, 32], [1, 32]]))
                # V^T gather from w-major pw rows: vts[p, w*32+h] = pw[w+p, (h,w)]
                vts = vtsp.tile([32, L], BF16, tag="vts", name="vts")
                nc.sync.dma_start(
                    vts[:],
                    _cap(phd[:], [[1024, 32], [1056, 32], [1, 32]], 64 * 1024))
                # unpermute w-major -> natural into qa[96:128]
                nc.gpsimd.tensor_copy(
                    _cap(qa[96:128, :], [[L, 32], [32, 32], [1, 32]]),
                    _cap(vts[:], [[L, 32], [1, 32], [32, 32]]))

            def prep_qkri(n, qk_tag, phw_tag):
                ctx = prep_begin(n, qk_tag, phw_tag)
                for f in ctx["pe"][:7]:
                    f()
                rope_half(ctx, 0)
                for f in ctx["pe"][7:14]:
                    f()
                rope_half(ctx, 1)
                return ctx

            def prep_phw_end(ctx):
                ctx["pe"][14]()
                prep_copy(ctx, 0)
                ctx["pe"][15]()
                prep_copy(ctx, 1)
                prep_end(ctx)

            # ---------------- v phase (pairs of m-tiles; mm+pv rings give a
            # 4-deep pipe since attention hasn't started yet) ----
            def v_pass(groups, wtiles, sink):
                slots = [ps_tile("mm" if i % 2 == 0 else "pv", f"vs{i}")
                         for i in range(len(groups))]
                for k in range(6):
                    for (m, ch), slot in zip(groups, slots):
                        c0, cw = (0, 512) if ch == 0 else (512, 256)
                        nc.tensor.matmul(
                            slot[:, 0:cw],
                            xts[k][:, m * 128 : (m + 1) * 128],
                            wtiles[k][:, c0 : c0 + cw],
                            start=(k == 0), stop=(k == 5))
                for (m, ch), slot in zip(groups, slots):
                    sink(m, ch, slot)

            def va_sink(m, ch, slot):
                if ch == 0:
                    nc.scalar.activation(
                        _cap(vaug[m][:], [[780, 128], [65, 8], [1, 64]]),
                        slot[:, 0:512], AF.Copy)
                else:
                    nc.scalar.activation(
                        _cap(vaug[m][:], [[780, 128], [65, 4], [1, 64]], 65 * 8),
                        slot[:, 0:256], AF.Copy)

            vp = ([[(m, 0), (m + 1, 0)] for m in (0, 2, 4, 6)]
                  + [[(m, 1), (m + 1, 1)] for m in (0, 2, 4, 6)])
            preps = {}
            v_pass(vp[0], vws, va_sink)
            v_pass(vp[1], vws, va_sink)
            p0 = prep_qkri(0, "s", "pv")
            v_pass(vp[2], vws, va_sink)
            v_pass(vp[3], vws, va_sink)
            prep_phw_end(p0)
            p1 = prep_qkri(1, "s", "pv")
            v_pass(vp[4], vws, va_sink)
            v_pass(vp[5], vws, va_sink)
            prep_phw_end(p1)
            preps[2] = prep_qkri(2, "s", "mm")
            v_pass(vp[6], vws, va_sink)
            v_pass(vp[7], vws, va_sink)

            # ---------------- attention head loop (8 chunks per head) -------
            def S_pair(n, jt):
                sb = ps_big("sb")
                for ih in range(2):
                    nc.tensor.matmul(
                        sb[:, ih * 512 : (ih + 1) * 512],
                        ka_t[n][:, jt * 128 : (jt + 1) * 128],
                        qa_t[n][:, ih * 512 : (ih + 1) * 512],
                        start=True, stop=True)
                return sb

            def normalize(n, ih, pvt):
                last = n == NH - 1
                # evacuate PSUM immediately (frees the pv bank for the next
                # head); the rest runs from SBUF
                src = pvcp.tile([65, 512], F32, tag="pvc", name="pvc")
                nc.vector.tensor_copy(src[:], pvt[0:65, :])
                out_ap = outt[n // 2][(n % 2) * 64 : (n % 2) * 64 + 64,
                                      ih * 512 : (ih + 1) * 512]
                if last:
                    # tail latency path: broadcast the reciprocal row via a
                    # K=1 matmul instead of the DRAM round-trip, multiply on
                    # DVE (reads one PSUM operand); bf16 so dtypes match the
                    # ones row
                    recb = recp.tile([1, 512], BF16, tag="recb", name="recb")
                    with nc.allow_low_precision(reason="bf16 recip broadcast"):
                        nc.vector.reciprocal(recb[:], src[64:65, :])
                    recP = ps_tile("mm", "recP")
                    nc.tensor.matmul(recP[0:64, :], cs(ONES, ONES + 64),
                                     recb[:], start=True, stop=True)
                    nc.vector.tensor_mul(out_ap, src[0:64, :], recP[0:64, :])
                else:
                    rec = recp.tile([1, 512], F32, tag="rec", name="rec")
                    nc.vector.reciprocal(rec[:], src[64:65, :])
                    # partition-broadcast via DRAM bounce, issued on the ACT
                    # queue (SEQ-side cost only; keeps the SP queue clear for
                    # the gather chain)
                    recd = dram.tile([1, 512], F32, tag="recd", bufs=4,
                                     name="recd")
                    nc.sync.dma_start(recd[:], rec[:])
                    rec64 = rec64p.tile([64, 512], F32, tag="rec64",
                                        name="rec64")
                    nc.sync.dma_start(rec64[:], _cap(recd[:],
                                                     [[0, 64], [1, 512]]))
                    nc.gpsimd.tensor_mul(out_ap, src[0:64, :], rec64[:])

            pending = S_pair(0, 0)
            pvt = [None, None]
            pts = []
            for n in range(NH):
                if n in (8, 9, 10):
                    # prefetch proj weights (2 per head) while attention
                    # finishes
                    for k in range(2 * (n - 8), 2 * (n - 8) + 2):
                        t = vwp.tile([128, DIM], BF16, tag=f"vw{k}")
                        nc.sync.dma_start(t[:], P_t[k * 128 : (k + 1) * 128, :])
                        pts.append(t)
                ctx_qk = preps[n + 3] = prep_begin(n + 3, "mm", "mm") \
                    if n + 3 < NH else None
                ctx_ph = preps.get(n + 2)
                for jt in range(8):
                    sb = pending
                    ej = ejp.tile([128, 1024], BF16, tag="ej", name="ej")
                    nc.scalar.activation(ej[:], sb[:], AF.Exp, scale=0.125)
                    if jt < 7:
                        pending = S_pair(n, jt + 1)
                    elif n + 1 < NH:
                        pending = S_pair(n + 1, 0)
                    if jt == 0:
                        pvt[0] = ps_tile("pv", "pvA")
                        pvt[1] = ps_tile("pv", "pvB")
                    for ih in range(2):
                        nc.tensor.matmul(
                            pvt[ih][0:65, :],
                            vaug[jt][:, 65 * n : 65 * n + 65],
                            ej[:, ih * 512 : (ih + 1) * 512],
                            start=(jt == 0), stop=(jt == 7))
                    if ctx_qk is not None and jt <= 3:
                        # 4 qk matmuls per chunk -> both halves stopped by jt3
                        for i in range(4 * jt, min(4 * jt + 4, 14)):
                            ctx_qk["pe"][i]()
                        if jt == 1:
                            rope_half(ctx_qk, 0)
                        if jt == 3:
                            rope_half(ctx_qk, 1)
                    if jt == 4 and ctx_ph is not None:
                        ctx_ph["pe"][14]()
                        prep_copy(ctx_ph, 0)
                    if jt == 6 and ctx_ph is not None:
                        ctx_ph["pe"][15]()
                        prep_copy(ctx_ph, 1)
                    if jt == 7:
                        normalize(n, 0, pvt[0])
                        normalize(n, 1, pvt[1])
                        if ctx_ph is not None:
                            prep_end(ctx_ph)

            # ---------------- proj (alternate s/mm rings for 4-deep pipe) ---
            for m in range(8):
                ob = obp.tile([128, DIM], BF16, tag="ob", name="ob")
                if m % 3 == 0:
                    big = ps_big("prs")
                    slots = [big[:, 0:512], big[:, 512:768]]
                elif m % 3 == 1:
                    slots = [ps_tile("mm", "prsA")[:, 0:512],
                             ps_tile("mm", "prsA")[:, 0:256]]
                else:
                    slots = [ps_tile("pv", "prsB")[:, 0:512],
                             ps_tile("pv", "prsB")[:, 0:256]]
                for ch in range(2):
                    c0, cw = (0, 512) if ch == 0 else (512, 256)
                    slot = slots[ch]
                    for k in range(6):
                        nc.tensor.matmul(
                            slot,
                            outt[k][:, m * 128 : (m + 1) * 128],
                            pts[k][:, c0 : c0 + cw],
                            start=(k == 0), stop=False)
                    nc.tensor.matmul(slot, cs(ONES, ONES + 128),
                                     cs(PB + c0, PB + c0 + cw),
                                     start=False, stop=True)
                    # evacuate the two chunks on different engines (both are
                    # otherwise idle in the proj phase) so they overlap
                    if ch == 0:
                        nc.scalar.activation(ob[:, c0 : c0 + cw], slot,
                                             AF.Copy)
                    else:
                        nc.vector.tensor_copy(ob[:, c0 : c0 + cw], slot)
                nc.sync.dma_start(outD[m * 128 : (m + 1) * 128, :], ob[:])

        est.close()
    return nc


BEST_FLAGS = dict()

_BUILT = None


def _get_built():
    global _BUILT
    if _BUILT is None:
        _BUILT = build_bass(1, **BEST_FLAGS)
    return _BUILT


def _ensure_axon():
    """Re-enable the axon backend if the caller pinned JAX_PLATFORMS=cpu
    (common in reference harnesses)."""
    import jax

    def has_axon():
        try:
            return any(getattr(d, "platform", "") == "axon" or "NC_" in str(d)
                       for d in jax.devices())
        except Exception:
            return False

    if has_axon():
        return
    os.environ.pop("JAX_PLATFORMS", None)
    try:
        jax.config.update("jax_platforms", None)
    except Exception:
        pass
    try:
        from jax._src import xla_bridge
        xla_bridge._clear_backends()
    except Exception:
        pass
    assert has_axon(), "axon/neuron devices not visible to jax"


def kernel(x, qkv_w, qkv_b, proj_w, proj_b, rel_pos_h, rel_pos_w):
    _ensure_axon()
    x = np.asarray(x, np.float32)
    B = x.shape[0]
    in_maps = make_inmaps(x, qkv_w, qkv_b, proj_w, proj_b,
                          rel_pos_h, rel_pos_w)
    nc = _get_built()
    res = run_bass_kernel_spmd(nc, in_maps, list(range(B))).results
    out = np.stack([res[b]["out"] for b in range(B)])  # (B, 1024, 768)
    return np.ascontiguousarray(out.reshape(B, HH, WW, DIM).astype(np.float32))
